# revision 7
# baseline (speedup 1.0000x reference)
"""GNN message-passing kernel for 8 Trainium2 NeuronCores.

Math (see reference):
  out[e] = relu(BN_E(local[e] + global[e]))
  local[e]  = emb_src[feat[src_e]] @ We0 + emb_dst[feat[dst_e]] @ We1 + b_edge
  global[e] = (P1[src_e] @ P2[dst_e]) @ W3 + b3,  P1 = (h@W1+b1).reshape(N,H,H),
              P2 = h@W2+b2

Device strategy (edge-parallel over 8 cores, 40000 edges/core):
  - W3 is folded into W1 host-side: W1f[i, m*32+d] = sum_k W1[i,k*32+d] W3[k,m].
    The b1/b3 terms fold into a per-dst-node vector: P2B = P2@Btil + b3.
  - Host builds two per-core COMPACTED node tables (so gather indices fit
    int16 for the fast dma_gather instruction):
      HcatC[r] = [h[n] | emb_src[feat[n]]@We0 + b_edge/2]        n = uniq_src[r]
      PcatC[r] = [P2[n] | emb_dst[feat[n]]@We1 + b_edge/2 + P2B[n]]  uniq_dst[r]
  - Edge gathers use nc.gpsimd.dma_gather (InstDMAGatherAnt): 4096 edges per
    call, 256B rows, descriptors generated by the SWDGE CounterMachine
    (~0.34ns/desc) instead of the ~1us/row indirect-DMA path.
  - Per 128-edge tile on device:
      PE: Qt = transpose(h_src-tile); T1 = Qt.T @ W1f   [128, 1024]
      DVE: z = T1 * broadcast(P2_dst);  g = segment-reduce_d(z)  [128, 32]
      g += local terms;  PE accumulates sum/sumsq via ones-matmul
  - AllReduce (8 cores) of [sum|sumsq], BN scale/bias broadcast via PE,
    second pass normalize+relu, write out.
"""

import os
import numpy as np

H = 32
N = 40000
E = 320000
NCORES = 8
EC = E // NCORES          # 40000 edges per core
CH = 1024                 # edges per dma_gather chunk (HW limit ~1024 idxs)
NCH = 40                  # chunks per core
ECP = CH * NCH            # 40960 padded edges per core
TPC = ECP // 128          # 320 tiles of 128 edges
PAD = ECP - EC            # 960 dummy edges per core
TCAP = 32768              # compacted node-table capacity (int16 index range)
EPS = 1e-5

_cache = {}
last_exec_time_ns = None
last_results = None


def _build():
    if "nc" in _cache:
        return _cache["nc"]

    import concourse.bacc as bacc
    import concourse.bass as bass
    import concourse.mybir as mybir
    import concourse.tile as tile
    from concourse.masks import make_identity

    f32 = mybir.dt.float32
    i16 = mybir.dt.int16
    AF = mybir.ActivationFunctionType
    OP = mybir.AluOpType

    nc = bacc.Bacc("TRN2", target_bir_lowering=False, debug=False,
                   num_devices=NCORES)

    HCATC = nc.dram_tensor("hcatc", [TCAP, 64], f32, kind="ExternalInput").ap()
    PCATC = nc.dram_tensor("pcatc", [TCAP, 64], f32, kind="ExternalInput").ap()
    W1F = nc.dram_tensor("w1f", [H, H * H], f32, kind="ExternalInput").ap()
    SIDX = nc.dram_tensor("sidx", [128, ECP // 16], i16,
                          kind="ExternalInput").ap()
    DIDX = nc.dram_tensor("didx", [128, ECP // 16], i16,
                          kind="ExternalInput").ap()
    GB = nc.dram_tensor("gb", [1, 64], f32, kind="ExternalInput").ap()
    CORR = nc.dram_tensor("corr", [1, 64], f32, kind="ExternalInput").ap()
    OUT = nc.dram_tensor("out", [128, TPC * H], f32, kind="ExternalOutput").ap()

    OB = 32  # tiles per output write batch
    TPCH = CH // 128  # tiles per gather chunk (32)

    with tile.TileContext(nc) as tc:
        with tc.tile_pool(name="const", bufs=1) as cpool, \
             tc.tile_pool(name="big", bufs=1) as bigpool, \
             tc.tile_pool(name="gath", bufs=2) as gpool, \
             tc.tile_pool(name="work", bufs=2) as wpool, \
             tc.tile_pool(name="pst1", bufs=2, space="PSUM") as pst1, \
             tc.tile_pool(name="psqt", bufs=2, space="PSUM") as psqt, \
             tc.tile_pool(name="psmisc", bufs=1, space="PSUM") as psmisc, \
             tc.tile_pool(name="dram", bufs=1, space="DRAM") as dpool:

            ident = cpool.tile([128, 128], f32)
            make_identity(nc, ident[:])
            w1f_s = cpool.tile([H, H * H], f32)
            nc.sync.dma_start(w1f_s[:], W1F[:])
            sidx_s = cpool.tile([128, ECP // 16], i16)
            nc.sync.dma_start(sidx_s[:], SIDX[:])
            didx_s = cpool.tile([128, ECP // 16], i16)
            nc.sync.dma_start(didx_s[:], DIDX[:])
            gb_s = cpool.tile([1, 64], f32)
            nc.sync.dma_start(gb_s[:], GB[:])
            corr_s = cpool.tile([1, 64], f32)
            nc.sync.dma_start(corr_s[:], CORR[:])
            ones_col = cpool.tile([128, 1], f32)
            nc.vector.memset(ones_col[:], 1.0)
            ones_row = cpool.tile([1, 128], f32)
            nc.vector.memset(ones_row[:], 1.0)

            raw = bigpool.tile([128, TPC * H], f32)       # raw pre-BN output
            ssacc = psmisc.tile([1, 64], f32, tag="ssacc")  # [sum | sumsq]

            # ---------------- pass 1: per-tile bilinear ----------------
            for c in range(NCH):
                ic0 = c * (CH // 16)
                hsch = gpool.tile([128, TPCH, 64], f32, tag="hsch")
                pdch = gpool.tile([128, TPCH, 64], f32, tag="pdch")
                nc.gpsimd.dma_gather(
                    hsch[:], HCATC[:], sidx_s[:, ic0:ic0 + CH // 16],
                    CH, CH, 64)
                nc.gpsimd.dma_gather(
                    pdch[:], PCATC[:], didx_s[:, ic0:ic0 + CH // 16],
                    CH, CH, 64)

                for u in range(TPCH):
                    t = c * TPCH + u
                    hs = hsch[:, u, :]
                    pd = pdch[:, u, :]

                    qt_p = psqt.tile([H, 128], f32, tag="qt")
                    nc.tensor.transpose(out=qt_p[:], in_=hs[:, 0:H],
                                        identity=ident[:])
                    qt = wpool.tile([H, 128], f32, tag="qts")
                    nc.scalar.copy(qt[:], qt_p[:])

                    t1 = pst1.tile([128, H * H], f32, tag="t1")
                    nc.tensor.matmul(out=t1[:, 0:512], lhsT=qt[:],
                                     rhs=w1f_s[:, 0:512], start=True, stop=True)
                    nc.tensor.matmul(out=t1[:, 512:1024], lhsT=qt[:],
                                     rhs=w1f_s[:, 512:1024], start=True,
                                     stop=True)

                    z = wpool.tile([128, H * H], f32, tag="z")
                    pd_b = pd[:, 0:H].unsqueeze(1).to_broadcast([128, H, H])
                    nc.vector.tensor_tensor(
                        out=z[:], in0=t1[:].rearrange("p (m d) -> p m d", d=H),
                        in1=pd_b, op=OP.mult)

                    g = raw[:, t * H:(t + 1) * H]
                    nc.vector.tensor_reduce(
                        out=g, in_=z[:].rearrange("p (m d) -> p m d", d=H),
                        axis=mybir.AxisListType.X, op=OP.add)
                    lsum = wpool.tile([128, H], f32, tag="lsum")
                    nc.vector.tensor_tensor(out=lsum[:], in0=hs[:, H:2 * H],
                                            in1=pd[:, H:2 * H], op=OP.add)
                    nc.vector.tensor_tensor(out=g, in0=g, in1=lsum[:],
                                            op=OP.add)

                    sq = wpool.tile([128, H], f32, tag="sq")
                    nc.scalar.square(sq[:], g)
                    nc.tensor.matmul(out=ssacc[:, 0:H], lhsT=ones_col[:], rhs=g,
                                     start=(t == 0), stop=(t == TPC - 1),
                                     skip_group_check=True)
                    nc.tensor.matmul(out=ssacc[:, H:2 * H], lhsT=ones_col[:],
                                     rhs=sq[:],
                                     start=(t == 0), stop=(t == TPC - 1),
                                     skip_group_check=True)

            # ---------------- stats allreduce + BN coefficients --------
            stats = cpool.tile([1, 64], f32)
            nc.scalar.copy(stats[:], ssacc[:])
            cin = dpool.tile([1, 64], f32)
            cout = dpool.tile([1, 64], f32)
            nc.sync.dma_start(cin[:], stats[:])
            nc.gpsimd.collective_compute(
                "AllReduce", OP.add,
                replica_groups=[list(range(NCORES))],
                ins=[cin.opt()], outs=[cout.opt()])
            gstats = cpool.tile([1, 64], f32)
            nc.sync.dma_start(gstats[:], cout[:])

            mv = cpool.tile([1, 64], f32)
            nc.vector.tensor_tensor(out=mv[:], in0=gstats[:], in1=corr_s[:],
                                    op=OP.subtract)
            nc.vector.tensor_scalar_mul(mv[:], mv[:], 1.0 / E)
            var = cpool.tile([1, H], f32)
            nc.vector.tensor_tensor(out=var[:], in0=mv[:, 0:H],
                                    in1=mv[:, 0:H], op=OP.mult)
            nc.vector.tensor_tensor(out=var[:], in0=mv[:, H:2 * H],
                                    in1=var[:], op=OP.subtract)
            nc.vector.tensor_scalar_add(var[:], var[:], EPS)
            sd = cpool.tile([1, H], f32)
            nc.scalar.activation(sd[:], var[:], AF.Sqrt)
            rs = cpool.tile([1, H], f32)
            nc.vector.reciprocal(rs[:], sd[:])

            scaleb = cpool.tile([1, 64], f32)
            nc.vector.tensor_tensor(out=scaleb[:, 0:H], in0=gb_s[:, 0:H],
                                    in1=rs[:], op=OP.mult)
            tmp1 = cpool.tile([1, H], f32)
            nc.vector.tensor_tensor(out=tmp1[:], in0=mv[:, 0:H],
                                    in1=scaleb[:, 0:H], op=OP.mult)
            nc.vector.tensor_tensor(out=scaleb[:, H:2 * H], in0=gb_s[:, H:2 * H],
                                    in1=tmp1[:], op=OP.subtract)

            sb_p = psmisc.tile([128, 64], f32, tag="sbp")
            nc.tensor.matmul(out=sb_p[:], lhsT=ones_row[:], rhs=scaleb[:],
                             start=True, stop=True, skip_group_check=True)
            sb = cpool.tile([128, 64], f32)
            nc.scalar.copy(sb[:], sb_p[:])

            # ---------------- pass 2: normalize + relu -----------------
            for b0 in range(0, TPC, OB):
                nb = min(OB, TPC - b0)
                ob = wpool.tile([128, OB * H], f32, tag="ob")
                for t in range(b0, b0 + nb):
                    g = raw[:, t * H:(t + 1) * H]
                    tmp = wpool.tile([128, H], f32, tag="n1")
                    nc.vector.tensor_tensor(out=tmp[:], in0=g,
                                            in1=sb[:, 0:H], op=OP.mult)
                    nc.vector.tensor_tensor(out=tmp[:], in0=tmp[:],
                                            in1=sb[:, H:2 * H], op=OP.add)
                    j = t - b0
                    nc.scalar.activation(ob[:, j * H:(j + 1) * H], tmp[:],
                                         AF.Relu)
                nc.sync.dma_start(OUT[:, b0 * H:(b0 + nb) * H],
                                  ob[:, 0:nb * H])

    nc.compile()
    _cache["nc"] = nc
    return nc


def _run_sim(nc, in_maps):
    """Local CoreSim validation path (no hardware): executes the kernel in
    the multi-core interpreter, returns a result object like the HW path."""
    import numpy as np
    from concourse.bass_interp import MultiCoreSim
    from concourse import bass_utils, mybir

    sim = MultiCoreSim(nc, num_cores=NCORES, num_workers=NCORES)
    for c in range(NCORES):
        core = sim.cores[c]
        for name, val in in_maps[c].items():
            core.tensor(name)[:] = val
        if nc.partition_id_tensor is not None:
            core.tensor(nc.partition_id_tensor.name)[:] = np.array(
                [[c]], dtype=np.uint32)
    sim.simulate()
    results = []
    for c in range(NCORES):
        outs = {}
        for alloc in nc.m.functions[0].allocations:
            if isinstance(alloc, mybir.MemoryLocationSet) and \
                    alloc.kind == "ExternalOutput":
                name = alloc.memorylocations[0].name
                outs[name] = np.array(sim.cores[c].tensor(name))
        results.append(outs)
    return bass_utils.BassKernelResults(
        results=results, instructions_and_trace=None, profile_json=None,
        exec_time_ns=None)


def _prep_idx16(inv):
    """Wrap int16 indices into the dma_gather layout: idx i at
    [i % 16, i // 16], replicated across the 8 16-partition groups."""
    pad = np.zeros(ECP, dtype=np.int16)
    pad[:EC] = inv
    w = np.ascontiguousarray(pad.reshape(ECP // 16, 16).T)   # [16, ECP//16]
    return np.ascontiguousarray(np.tile(w, (8, 1)))          # [128, ECP//16]


def kernel(h, e, feat, src_idx, dst_idx, emb_src, emb_dst, W_edge, b_edge,
           W1, b1, W2, b2, W3, b3, gamma, beta):
    global last_exec_time_ns, last_results
    import concourse.bass_utils as bass_utils

    h = np.asarray(h, np.float32)
    feat = np.asarray(feat, np.int64)
    src_idx = np.asarray(src_idx, np.int64)
    dst_idx = np.asarray(dst_idx, np.int64)
    emb_src = np.asarray(emb_src, np.float32)
    emb_dst = np.asarray(emb_dst, np.float32)
    W_edge = np.asarray(W_edge, np.float32)
    b_edge = np.asarray(b_edge, np.float32)
    W1 = np.asarray(W1, np.float32)
    b1 = np.asarray(b1, np.float32)
    W2 = np.asarray(W2, np.float32)
    b2 = np.asarray(b2, np.float32)
    W3 = np.asarray(W3, np.float32)
    b3 = np.asarray(b3, np.float32)
    gamma = np.asarray(gamma, np.float32)
    beta = np.asarray(beta, np.float32)

    # ---- host-side weight folds and node tables ----
    ES = emb_src @ W_edge[:H] + 0.5 * b_edge              # [V, H]
    ED = emb_dst @ W_edge[H:] + 0.5 * b_edge
    W1r = W1.reshape(H, H, H)                             # [i, k, d]
    W1f = np.ascontiguousarray(
        np.einsum("ikd,km->imd", W1r, W3).reshape(H, H * H)).astype(np.float32)
    Btil = np.einsum("kd,km->dm", b1.reshape(H, H), W3)   # [d, m]
    P2 = h @ W2 + b2                                      # [N, H]
    P2B = P2 @ Btil + b3                                  # [N, H]
    Hcat = np.ascontiguousarray(
        np.concatenate([h, ES[feat]], axis=1)).astype(np.float32)
    Pcat = np.ascontiguousarray(
        np.concatenate([P2, ED[feat] + P2B], axis=1)).astype(np.float32)

    gb = np.concatenate([gamma, beta]).reshape(1, 64).astype(np.float32)

    nc = _build()

    # per-core compacted tables + int16 indices + exact BN pad correction
    in_maps = []
    corr_sum = np.zeros(H, np.float64)
    corr_sq = np.zeros(H, np.float64)
    W1f3 = W1f.reshape(H, H, H)                           # [i, m, d]
    per_core = []
    for c in range(NCORES):
        sl = slice(c * EC, (c + 1) * EC)
        su, sinv = np.unique(src_idx[sl], return_inverse=True)
        du, dinv = np.unique(dst_idx[sl], return_inverse=True)
        assert len(su) <= TCAP and len(du) <= TCAP, (len(su), len(du))
        HcatC = np.zeros((TCAP, 64), np.float32)
        HcatC[:len(su)] = Hcat[su]
        PcatC = np.zeros((TCAP, 64), np.float32)
        PcatC[:len(du)] = Pcat[du]
        per_core.append((HcatC, PcatC, sinv.astype(np.int16),
                         dinv.astype(np.int16)))
        # dummy padded edge (table rows 0, 0) contribution to BN stats
        v = np.einsum("i,imd,d->m", Hcat[su[0], :H].astype(np.float64),
                      W1f3.astype(np.float64),
                      Pcat[du[0], :H].astype(np.float64)) \
            + Hcat[su[0], H:] + Pcat[du[0], H:]
        corr_sum += PAD * v
        corr_sq += PAD * v * v

    corr = np.zeros((1, 64), np.float32)
    corr[0, :H] = corr_sum
    corr[0, H:] = corr_sq

    for c in range(NCORES):
        HcatC, PcatC, sinv, dinv = per_core[c]
        in_maps.append({
            "hcatc": HcatC,
            "pcatc": PcatC,
            "w1f": W1f,
            "sidx": _prep_idx16(sinv),
            "didx": _prep_idx16(dinv),
            "gb": gb,
            "corr": corr,
        })

    _cache["last_in_maps"] = in_maps
    if os.environ.get("KERNEL_SIM", "0") == "1":
        res = _run_sim(nc, in_maps)
    else:
        trace = bool(int(os.environ.get("KERNEL_TRACE", "0")))
        res = bass_utils.run_bass_kernel_spmd(
            nc, in_maps, core_ids=list(range(NCORES)), trace=trace)
    last_results = res
    last_exec_time_ns = res.exec_time_ns

    outs = []
    for c in range(NCORES):
        o = res.results[c]["out"].reshape(128, TPC, H)
        outs.append(o.transpose(1, 0, 2).reshape(ECP, H)[:EC])
    return np.ascontiguousarray(np.concatenate(outs, axis=0))


# revision 13
# speedup vs baseline: 1.0131x; 1.0131x over previous
"""GNN message-passing kernel for 8 Trainium2 NeuronCores.

Math (see reference):
  out[e] = relu(BN_E(local[e] + global[e]))
  local[e]  = emb_src[feat[src_e]] @ We0 + emb_dst[feat[dst_e]] @ We1 + b_edge
  global[e] = (P1[src_e] @ P2[dst_e]) @ W3 + b3,  P1 = (h@W1+b1).reshape(N,H,H),
              P2 = h@W2+b2

Device strategy (edge-parallel over 8 cores, 40000 edges/core):
  - W3 is folded into W1 host-side: W1f[i, m*32+d] = sum_k W1[i,k*32+d] W3[k,m].
    The b1/b3 terms fold into a per-dst-node vector: P2B = P2@Btil + b3.
  - Host builds two per-core COMPACTED node tables (so gather indices fit
    int16 for the fast dma_gather instruction):
      HcatC[r] = [h[n] | emb_src[feat[n]]@We0 + b_edge/2]        n = uniq_src[r]
      PcatC[r] = [P2[n] | emb_dst[feat[n]]@We1 + b_edge/2 + P2B[n]]  uniq_dst[r]
  - Edge gathers use nc.gpsimd.dma_gather (InstDMAGatherAnt): 4096 edges per
    call, 256B rows, descriptors generated by the SWDGE CounterMachine
    (~0.34ns/desc) instead of the ~1us/row indirect-DMA path.
  - Per 128-edge tile on device:
      PE: Qt = transpose(h_src-tile); T1 = Qt.T @ W1f   [128, 1024]
      DVE: z = T1 * broadcast(P2_dst);  g = segment-reduce_d(z)  [128, 32]
      g += local terms;  PE accumulates sum/sumsq via ones-matmul
  - AllReduce (8 cores) of [sum|sumsq], BN scale/bias broadcast via PE,
    second pass normalize+relu, write out.
"""

import os
import numpy as np

H = 32
N = 40000
E = 320000
NCORES = 8
EC = E // NCORES          # 40000 edges per core
CH = 1024                 # edges per dma_gather chunk (HW limit ~1024 idxs)
NCH = 40                  # chunks per core
ECP = CH * NCH            # 40960 padded edges per core
TPC = ECP // 128          # 320 tiles of 128 edges
PAD = ECP - EC            # 960 dummy edges per core
TCAP = 32768              # compacted node-table capacity (int16 index range)
EPS = 1e-5

_cache = {}
last_exec_time_ns = None
last_results = None


def _build():
    if "nc" in _cache:
        return _cache["nc"]
    variant = os.environ.get("KERNEL_VARIANT", "")
    do_gather = variant != "computeonly"
    do_compute = variant != "gatheronly"

    import concourse.bacc as bacc
    import concourse.bass as bass
    import concourse.mybir as mybir
    import concourse.tile as tile
    from concourse.masks import make_identity

    f32 = mybir.dt.float32
    i16 = mybir.dt.int16
    AF = mybir.ActivationFunctionType
    OP = mybir.AluOpType

    nc = bacc.Bacc("TRN2", target_bir_lowering=False, debug=False,
                   num_devices=NCORES)

    HCATC = nc.dram_tensor("hcatc", [TCAP, 64], f32, kind="ExternalInput").ap()
    PCATC = nc.dram_tensor("pcatc", [TCAP, 64], f32, kind="ExternalInput").ap()
    W1F = nc.dram_tensor("w1f", [H, H * H], f32, kind="ExternalInput").ap()
    SIDX = nc.dram_tensor("sidx", [128, ECP // 16], i16,
                          kind="ExternalInput").ap()
    DIDX = nc.dram_tensor("didx", [128, ECP // 16], i16,
                          kind="ExternalInput").ap()
    GB = nc.dram_tensor("gb", [1, 64], f32, kind="ExternalInput").ap()
    CORR = nc.dram_tensor("corr", [1, 64], f32, kind="ExternalInput").ap()
    OUT = nc.dram_tensor("out", [128, TPC * H], f32, kind="ExternalOutput").ap()

    OB = 32  # tiles per output write batch
    TPCH = CH // 128  # tiles per gather chunk (32)

    with tile.TileContext(nc) as tc:
        with tc.tile_pool(name="const", bufs=1) as cpool, \
             tc.tile_pool(name="big", bufs=1) as bigpool, \
             tc.tile_pool(name="gath", bufs=2) as gpool, \
             tc.tile_pool(name="work", bufs=2) as wpool, \
             tc.tile_pool(name="pst1", bufs=2, space="PSUM") as pst1, \
             tc.tile_pool(name="psqt", bufs=2, space="PSUM") as psqt, \
             tc.tile_pool(name="psmisc", bufs=1, space="PSUM") as psmisc, \
             tc.tile_pool(name="dram", bufs=1, space="DRAM") as dpool:

            ident = cpool.tile([128, 128], f32)
            make_identity(nc, ident[:])
            w1f_s = cpool.tile([H, H * H], f32)
            nc.sync.dma_start(w1f_s[:], W1F[:])
            sidx_s = cpool.tile([128, ECP // 16], i16)
            nc.sync.dma_start(sidx_s[:], SIDX[:])
            didx_s = cpool.tile([128, ECP // 16], i16)
            nc.sync.dma_start(didx_s[:], DIDX[:])
            gb_s = cpool.tile([1, 64], f32)
            nc.sync.dma_start(gb_s[:], GB[:])
            corr_s = cpool.tile([1, 64], f32)
            nc.sync.dma_start(corr_s[:], CORR[:])
            ones_col = cpool.tile([128, 1], f32)
            nc.vector.memset(ones_col[:], 1.0)
            ones_row = cpool.tile([1, 128], f32)
            nc.vector.memset(ones_row[:], 1.0)

            raw = bigpool.tile([128, TPC * H], f32)       # raw pre-BN output
            ssacc = psmisc.tile([1, 64], f32, tag="ssacc")  # [sum | sumsq]
            if not do_compute:
                nc.vector.memset(raw[:], 0.0)

            # ---------------- pass 1: per-tile bilinear ----------------
            for c in range(NCH):
                ic0 = c * (CH // 16)
                hsch = gpool.tile([128, TPCH, 64], f32, tag="hsch")
                pdch = gpool.tile([128, TPCH, 64], f32, tag="pdch")
                if do_gather:
                    nc.gpsimd.dma_gather(
                        hsch[:], HCATC[:], sidx_s[:, ic0:ic0 + CH // 16],
                        CH, CH, 64)
                    nc.gpsimd.dma_gather(
                        pdch[:], PCATC[:], didx_s[:, ic0:ic0 + CH // 16],
                        CH, CH, 64)
                else:
                    nc.vector.memset(hsch[:], 0.125)
                    nc.vector.memset(pdch[:], 0.125)

                for u in range(TPCH if do_compute else (1 if c == 0 else 0)):
                    t = c * TPCH + u
                    hs = hsch[:, u, :]
                    pd = pdch[:, u, :]

                    qt_p = psqt.tile([H, 128], f32, tag="qt")
                    nc.tensor.transpose(out=qt_p[:], in_=hs[:, 0:H],
                                        identity=ident[:])
                    qt = wpool.tile([H, 128], f32, tag="qts")
                    nc.scalar.copy(qt[:], qt_p[:])

                    t1 = pst1.tile([128, H * H], f32, tag="t1")
                    nc.tensor.matmul(out=t1[:, 0:512], lhsT=qt[:],
                                     rhs=w1f_s[:, 0:512], start=True, stop=True)
                    nc.tensor.matmul(out=t1[:, 512:1024], lhsT=qt[:],
                                     rhs=w1f_s[:, 512:1024], start=True,
                                     stop=True)

                    z = wpool.tile([128, H * H], f32, tag="z")
                    pd_b = pd[:, 0:H].unsqueeze(1).to_broadcast([128, H, H])
                    nc.vector.tensor_tensor(
                        out=z[:], in0=t1[:].rearrange("p (m d) -> p m d", d=H),
                        in1=pd_b, op=OP.mult)

                    g = raw[:, t * H:(t + 1) * H]
                    nc.vector.tensor_reduce(
                        out=g, in_=z[:].rearrange("p (m d) -> p m d", d=H),
                        axis=mybir.AxisListType.X, op=OP.add)
                    lsum = wpool.tile([128, H], f32, tag="lsum")
                    nc.vector.tensor_tensor(out=lsum[:], in0=hs[:, H:2 * H],
                                            in1=pd[:, H:2 * H], op=OP.add)
                    nc.vector.tensor_tensor(out=g, in0=g, in1=lsum[:],
                                            op=OP.add)

                    sq = wpool.tile([128, H], f32, tag="sq")
                    nc.scalar.square(sq[:], g)
                    last = (t == TPC - 1) if do_compute else True
                    nc.tensor.matmul(out=ssacc[:, 0:H], lhsT=ones_col[:], rhs=g,
                                     start=(t == 0), stop=last,
                                     skip_group_check=True)
                    nc.tensor.matmul(out=ssacc[:, H:2 * H], lhsT=ones_col[:],
                                     rhs=sq[:],
                                     start=(t == 0), stop=last,
                                     skip_group_check=True)

            # ---------------- stats allreduce + BN coefficients --------
            stats = cpool.tile([1, 64], f32)
            nc.scalar.copy(stats[:], ssacc[:])
            cin = dpool.tile([1, 64], f32)
            cout = dpool.tile([1, 64], f32)
            nc.sync.dma_start(cin[:], stats[:])
            nc.gpsimd.collective_compute(
                "AllReduce", OP.add,
                replica_groups=[list(range(NCORES))],
                ins=[cin.opt()], outs=[cout.opt()])
            gstats = cpool.tile([1, 64], f32)
            nc.sync.dma_start(gstats[:], cout[:])

            mv = cpool.tile([1, 64], f32)
            nc.vector.tensor_tensor(out=mv[:], in0=gstats[:], in1=corr_s[:],
                                    op=OP.subtract)
            nc.vector.tensor_scalar_mul(mv[:], mv[:], 1.0 / E)
            var = cpool.tile([1, H], f32)
            nc.vector.tensor_tensor(out=var[:], in0=mv[:, 0:H],
                                    in1=mv[:, 0:H], op=OP.mult)
            nc.vector.tensor_tensor(out=var[:], in0=mv[:, H:2 * H],
                                    in1=var[:], op=OP.subtract)
            nc.vector.tensor_scalar_add(var[:], var[:], EPS)
            sd = cpool.tile([1, H], f32)
            nc.scalar.activation(sd[:], var[:], AF.Sqrt)
            rs = cpool.tile([1, H], f32)
            nc.vector.reciprocal(rs[:], sd[:])

            scaleb = cpool.tile([1, 64], f32)
            nc.vector.tensor_tensor(out=scaleb[:, 0:H], in0=gb_s[:, 0:H],
                                    in1=rs[:], op=OP.mult)
            tmp1 = cpool.tile([1, H], f32)
            nc.vector.tensor_tensor(out=tmp1[:], in0=mv[:, 0:H],
                                    in1=scaleb[:, 0:H], op=OP.mult)
            nc.vector.tensor_tensor(out=scaleb[:, H:2 * H], in0=gb_s[:, H:2 * H],
                                    in1=tmp1[:], op=OP.subtract)

            sb_p = psmisc.tile([128, 64], f32, tag="sbp")
            nc.tensor.matmul(out=sb_p[:], lhsT=ones_row[:], rhs=scaleb[:],
                             start=True, stop=True, skip_group_check=True)
            sb = cpool.tile([128, 64], f32)
            nc.scalar.copy(sb[:], sb_p[:])

            # ---------------- pass 2: normalize + relu -----------------
            for b0 in range(0, TPC, OB):
                nb = min(OB, TPC - b0)
                ob = wpool.tile([128, OB * H], f32, tag="ob")
                for t in range(b0, b0 + nb):
                    g = raw[:, t * H:(t + 1) * H]
                    tmp = wpool.tile([128, H], f32, tag="n1")
                    nc.vector.tensor_tensor(out=tmp[:], in0=g,
                                            in1=sb[:, 0:H], op=OP.mult)
                    nc.vector.tensor_tensor(out=tmp[:], in0=tmp[:],
                                            in1=sb[:, H:2 * H], op=OP.add)
                    j = t - b0
                    nc.scalar.activation(ob[:, j * H:(j + 1) * H], tmp[:],
                                         AF.Relu)
                nc.sync.dma_start(OUT[:, b0 * H:(b0 + nb) * H],
                                  ob[:, 0:nb * H])

    nc.compile()
    _cache["nc"] = nc
    return nc


def _run_sim(nc, in_maps):
    """Local CoreSim validation path (no hardware): executes the kernel in
    the multi-core interpreter, returns a result object like the HW path."""
    import numpy as np
    from concourse.bass_interp import MultiCoreSim
    from concourse import bass_utils, mybir

    sim = MultiCoreSim(nc, num_cores=NCORES, num_workers=NCORES)
    for c in range(NCORES):
        core = sim.cores[c]
        for name, val in in_maps[c].items():
            core.tensor(name)[:] = val
        if nc.partition_id_tensor is not None:
            core.tensor(nc.partition_id_tensor.name)[:] = np.array(
                [[c]], dtype=np.uint32)
    sim.simulate()
    results = []
    for c in range(NCORES):
        outs = {}
        for alloc in nc.m.functions[0].allocations:
            if isinstance(alloc, mybir.MemoryLocationSet) and \
                    alloc.kind == "ExternalOutput":
                name = alloc.memorylocations[0].name
                outs[name] = np.array(sim.cores[c].tensor(name))
        results.append(outs)
    return bass_utils.BassKernelResults(
        results=results, instructions_and_trace=None, profile_json=None,
        exec_time_ns=None)


def _prep_idx16(inv):
    """Wrap int16 indices into the dma_gather layout: idx i at
    [i % 16, i // 16], replicated across the 8 16-partition groups."""
    pad = np.zeros(ECP, dtype=np.int16)
    pad[:EC] = inv
    w = np.ascontiguousarray(pad.reshape(ECP // 16, 16).T)   # [16, ECP//16]
    return np.ascontiguousarray(np.tile(w, (8, 1)))          # [128, ECP//16]


def kernel(h, e, feat, src_idx, dst_idx, emb_src, emb_dst, W_edge, b_edge,
           W1, b1, W2, b2, W3, b3, gamma, beta):
    global last_exec_time_ns, last_results
    import concourse.bass_utils as bass_utils

    h = np.asarray(h, np.float32)
    feat = np.asarray(feat, np.int64)
    src_idx = np.asarray(src_idx, np.int64)
    dst_idx = np.asarray(dst_idx, np.int64)
    emb_src = np.asarray(emb_src, np.float32)
    emb_dst = np.asarray(emb_dst, np.float32)
    W_edge = np.asarray(W_edge, np.float32)
    b_edge = np.asarray(b_edge, np.float32)
    W1 = np.asarray(W1, np.float32)
    b1 = np.asarray(b1, np.float32)
    W2 = np.asarray(W2, np.float32)
    b2 = np.asarray(b2, np.float32)
    W3 = np.asarray(W3, np.float32)
    b3 = np.asarray(b3, np.float32)
    gamma = np.asarray(gamma, np.float32)
    beta = np.asarray(beta, np.float32)

    # ---- host-side weight folds and node tables ----
    ES = emb_src @ W_edge[:H] + 0.5 * b_edge              # [V, H]
    ED = emb_dst @ W_edge[H:] + 0.5 * b_edge
    W1r = W1.reshape(H, H, H)                             # [i, k, d]
    W1f = np.ascontiguousarray(
        np.einsum("ikd,km->imd", W1r, W3).reshape(H, H * H)).astype(np.float32)
    Btil = np.einsum("kd,km->dm", b1.reshape(H, H), W3)   # [d, m]
    P2 = h @ W2 + b2                                      # [N, H]
    P2B = P2 @ Btil + b3                                  # [N, H]
    Hcat = np.ascontiguousarray(
        np.concatenate([h, ES[feat]], axis=1)).astype(np.float32)
    Pcat = np.ascontiguousarray(
        np.concatenate([P2, ED[feat] + P2B], axis=1)).astype(np.float32)

    gb = np.concatenate([gamma, beta]).reshape(1, 64).astype(np.float32)

    nc = _build()

    # per-core compacted tables + int16 indices + exact BN pad correction
    in_maps = []
    corr_sum = np.zeros(H, np.float64)
    corr_sq = np.zeros(H, np.float64)
    W1f3 = W1f.reshape(H, H, H)                           # [i, m, d]
    per_core = []
    for c in range(NCORES):
        sl = slice(c * EC, (c + 1) * EC)
        su, sinv = np.unique(src_idx[sl], return_inverse=True)
        du, dinv = np.unique(dst_idx[sl], return_inverse=True)
        assert len(su) <= TCAP and len(du) <= TCAP, (len(su), len(du))
        HcatC = np.zeros((TCAP, 64), np.float32)
        HcatC[:len(su)] = Hcat[su]
        PcatC = np.zeros((TCAP, 64), np.float32)
        PcatC[:len(du)] = Pcat[du]
        per_core.append((HcatC, PcatC, sinv.astype(np.int16),
                         dinv.astype(np.int16)))
        # dummy padded edge (table rows 0, 0) contribution to BN stats
        v = np.einsum("i,imd,d->m", Hcat[su[0], :H].astype(np.float64),
                      W1f3.astype(np.float64),
                      Pcat[du[0], :H].astype(np.float64)) \
            + Hcat[su[0], H:] + Pcat[du[0], H:]
        corr_sum += PAD * v
        corr_sq += PAD * v * v

    corr = np.zeros((1, 64), np.float32)
    corr[0, :H] = corr_sum
    corr[0, H:] = corr_sq

    for c in range(NCORES):
        HcatC, PcatC, sinv, dinv = per_core[c]
        in_maps.append({
            "hcatc": HcatC,
            "pcatc": PcatC,
            "w1f": W1f,
            "sidx": _prep_idx16(sinv),
            "didx": _prep_idx16(dinv),
            "gb": gb,
            "corr": corr,
        })

    _cache["last_in_maps"] = in_maps
    if os.environ.get("KERNEL_SIM", "0") == "1":
        res = _run_sim(nc, in_maps)
    else:
        trace = bool(int(os.environ.get("KERNEL_TRACE", "0")))
        res = bass_utils.run_bass_kernel_spmd(
            nc, in_maps, core_ids=list(range(NCORES)), trace=trace)
    last_results = res
    last_exec_time_ns = res.exec_time_ns

    outs = []
    for c in range(NCORES):
        o = res.results[c]["out"].reshape(128, TPC, H)
        outs.append(o.transpose(1, 0, 2).reshape(ECP, H)[:EC])
    return np.ascontiguousarray(np.concatenate(outs, axis=0))


# revision 17
# speedup vs baseline: 1.0372x; 1.0239x over previous
"""GNN message-passing kernel for 8 Trainium2 NeuronCores.

Math (see reference):
  out[e] = relu(BN_E(local[e] + global[e]))
  local[e]  = emb_src[feat[src_e]] @ We0 + emb_dst[feat[dst_e]] @ We1 + b_edge
  global[e] = (P1[src_e] @ P2[dst_e]) @ W3 + b3,  P1 = (h@W1+b1).reshape(N,H,H),
              P2 = h@W2+b2

Device strategy (edge-parallel over 8 cores, 40000 edges/core):
  - W3 is folded into W1 host-side: W1f[i, m*32+d] = sum_k W1[i,k*32+d] W3[k,m].
    The b1/b3 terms fold into a per-dst-node vector: P2B = P2@Btil + b3.
  - Host builds two per-core COMPACTED node tables (so gather indices fit
    int16 for the fast dma_gather instruction):
      HcatC[r] = [h[n] | emb_src[feat[n]]@We0 + b_edge/2]        n = uniq_src[r]
      PcatC[r] = [P2[n] | emb_dst[feat[n]]@We1 + b_edge/2 + P2B[n]]  uniq_dst[r]
  - Edge gathers use nc.gpsimd.dma_gather (InstDMAGatherAnt): 4096 edges per
    call, 256B rows, descriptors generated by the SWDGE CounterMachine
    (~0.34ns/desc) instead of the ~1us/row indirect-DMA path.
  - Per 128-edge tile on device:
      PE: Qt = transpose(h_src-tile); T1 = Qt.T @ W1f   [128, 1024]
      DVE: z = T1 * broadcast(P2_dst);  g = segment-reduce_d(z)  [128, 32]
      g += local terms;  PE accumulates sum/sumsq via ones-matmul
  - AllReduce (8 cores) of [sum|sumsq], BN scale/bias broadcast via PE,
    second pass normalize+relu, write out.
"""

import os
import numpy as np

H = 32
N = 40000
E = 320000
NCORES = 8
EC = E // NCORES          # 40000 edges per core
CH = 1024                 # edges per dma_gather chunk (HW limit ~1024 idxs)
NCH = 40                  # chunks per core
ECP = CH * NCH            # 40960 padded edges per core
TPC = ECP // 128          # 320 tiles of 128 edges
PAD = ECP - EC            # 960 dummy edges per core
TCAP = 32768              # compacted node-table capacity (int16 index range)
EPS = 1e-5

_cache = {}
last_exec_time_ns = None
last_results = None


def _build():
    if "nc" in _cache:
        return _cache["nc"]
    variant = os.environ.get("KERNEL_VARIANT", "")
    do_gather = variant != "computeonly"
    do_compute = variant != "gatheronly"
    repeat = int(os.environ.get("KERNEL_REPEAT", "1"))

    import concourse.bacc as bacc
    import concourse.bass as bass
    import concourse.mybir as mybir
    import concourse.tile as tile
    from concourse.masks import make_identity

    f32 = mybir.dt.float32
    i16 = mybir.dt.int16
    AF = mybir.ActivationFunctionType
    OP = mybir.AluOpType

    nc = bacc.Bacc("TRN2", target_bir_lowering=False, debug=False,
                   num_devices=NCORES)

    HCATC = nc.dram_tensor("hcatc", [TCAP, 64], f32, kind="ExternalInput").ap()
    PCATC = nc.dram_tensor("pcatc", [TCAP, 64], f32, kind="ExternalInput").ap()
    W1F = nc.dram_tensor("w1f", [H, H * H], f32, kind="ExternalInput").ap()
    SIDX = nc.dram_tensor("sidx", [128, ECP // 16], i16,
                          kind="ExternalInput").ap()
    DIDX = nc.dram_tensor("didx", [128, ECP // 16], i16,
                          kind="ExternalInput").ap()
    GB = nc.dram_tensor("gb", [1, 64], f32, kind="ExternalInput").ap()
    CORR = nc.dram_tensor("corr", [1, 64], f32, kind="ExternalInput").ap()
    OUT = nc.dram_tensor("out", [128, TPC * H], f32, kind="ExternalOutput").ap()

    OB = 32  # tiles per output write batch
    TPCH = CH // 128  # tiles per gather chunk (32)

    with tile.TileContext(nc) as tc:
        with tc.tile_pool(name="const", bufs=1) as cpool, \
             tc.tile_pool(name="big", bufs=1) as bigpool, \
             tc.tile_pool(name="gath", bufs=8) as gpool, \
             tc.tile_pool(name="work", bufs=2) as wpool, \
             tc.tile_pool(name="pst1", bufs=2, space="PSUM") as pst1, \
             tc.tile_pool(name="psqt", bufs=2, space="PSUM") as psqt, \
             tc.tile_pool(name="psmisc", bufs=1, space="PSUM") as psmisc, \
             tc.tile_pool(name="dram", bufs=1, space="DRAM") as dpool:

            ident = cpool.tile([128, 128], f32)
            make_identity(nc, ident[:])
            w1f_s = cpool.tile([H, H * H], f32)
            nc.sync.dma_start(w1f_s[:], W1F[:])
            sidx_s = cpool.tile([128, ECP // 16], i16)
            nc.sync.dma_start(sidx_s[:], SIDX[:])
            didx_s = cpool.tile([128, ECP // 16], i16)
            nc.sync.dma_start(didx_s[:], DIDX[:])
            gb_s = cpool.tile([1, 64], f32)
            nc.sync.dma_start(gb_s[:], GB[:])
            corr_s = cpool.tile([1, 64], f32)
            nc.sync.dma_start(corr_s[:], CORR[:])
            ones_col = cpool.tile([128, 1], f32)
            nc.vector.memset(ones_col[:], 1.0)
            ones_row = cpool.tile([1, 128], f32)
            nc.vector.memset(ones_row[:], 1.0)

            raw = bigpool.tile([128, TPC * H], f32)       # raw pre-BN output
            ssacc = psmisc.tile([1, 64], f32, tag="ssacc")  # [sum | sumsq]
            if not do_compute:
                nc.vector.memset(raw[:], 0.0)

            # ---------------- pass 1: per-tile bilinear ----------------
            for rep in range(repeat):
              for c in range(NCH):
                ic0 = c * (CH // 16)
                hsch = gpool.tile([128, TPCH, 64], f32, tag="hsch")
                pdch = gpool.tile([128, TPCH, 64], f32, tag="pdch")
                if do_gather:
                    nc.gpsimd.dma_gather(
                        hsch[:], HCATC[:], sidx_s[:, ic0:ic0 + CH // 16],
                        CH, CH, 64)
                    nc.gpsimd.dma_gather(
                        pdch[:], PCATC[:], didx_s[:, ic0:ic0 + CH // 16],
                        CH, CH, 64)
                else:
                    nc.vector.memset(hsch[:], 0.125)
                    nc.vector.memset(pdch[:], 0.125)

                for u in range(TPCH if do_compute else (1 if c == 0 else 0)):
                    t = c * TPCH + u
                    hs = hsch[:, u, :]
                    pd = pdch[:, u, :]

                    qt_p = psqt.tile([H, 128], f32, tag="qt")
                    nc.tensor.transpose(out=qt_p[:], in_=hs[:, 0:H],
                                        identity=ident[:])
                    qt = wpool.tile([H, 128], f32, tag="qts")
                    nc.scalar.copy(qt[:], qt_p[:])

                    t1 = pst1.tile([128, H * H], f32, tag="t1")
                    nc.tensor.matmul(out=t1[:, 0:512], lhsT=qt[:],
                                     rhs=w1f_s[:, 0:512], start=True, stop=True)
                    nc.tensor.matmul(out=t1[:, 512:1024], lhsT=qt[:],
                                     rhs=w1f_s[:, 512:1024], start=True,
                                     stop=True)

                    z = wpool.tile([128, H * H], f32, tag="z")
                    pd_b = pd[:, 0:H].unsqueeze(1).to_broadcast([128, H, H])
                    nc.vector.tensor_tensor(
                        out=z[:], in0=t1[:].rearrange("p (m d) -> p m d", d=H),
                        in1=pd_b, op=OP.mult)

                    g = raw[:, t * H:(t + 1) * H]
                    nc.vector.tensor_reduce(
                        out=g, in_=z[:].rearrange("p (m d) -> p m d", d=H),
                        axis=mybir.AxisListType.X, op=OP.add)
                    lsum = wpool.tile([128, H], f32, tag="lsum")
                    nc.vector.tensor_tensor(out=lsum[:], in0=hs[:, H:2 * H],
                                            in1=pd[:, H:2 * H], op=OP.add)
                    nc.vector.tensor_tensor(out=g, in0=g, in1=lsum[:],
                                            op=OP.add)

                    sq = wpool.tile([128, H], f32, tag="sq")
                    nc.scalar.square(sq[:], g)
                    last = (t == TPC - 1) if do_compute else True
                    nc.tensor.matmul(out=ssacc[:, 0:H], lhsT=ones_col[:], rhs=g,
                                     start=(t == 0), stop=last,
                                     skip_group_check=True)
                    nc.tensor.matmul(out=ssacc[:, H:2 * H], lhsT=ones_col[:],
                                     rhs=sq[:],
                                     start=(t == 0), stop=last,
                                     skip_group_check=True)

            # ---------------- stats allreduce + BN coefficients --------
            stats = cpool.tile([1, 64], f32)
            nc.scalar.copy(stats[:], ssacc[:])
            gstats = cpool.tile([1, 64], f32)
            if os.environ.get("KERNEL_1CORE", "0") == "1":
                nc.scalar.copy(gstats[:], stats[:])
            else:
                cin = dpool.tile([1, 64], f32)
                cout = dpool.tile([1, 64], f32)
                nc.sync.dma_start(cin[:], stats[:])
                nc.gpsimd.collective_compute(
                    "AllReduce", OP.add,
                    replica_groups=[list(range(NCORES))],
                    ins=[cin.opt()], outs=[cout.opt()])
                nc.sync.dma_start(gstats[:], cout[:])

            mv = cpool.tile([1, 64], f32)
            nc.vector.tensor_tensor(out=mv[:], in0=gstats[:], in1=corr_s[:],
                                    op=OP.subtract)
            nc.vector.tensor_scalar_mul(mv[:], mv[:], 1.0 / E)
            var = cpool.tile([1, H], f32)
            nc.vector.tensor_tensor(out=var[:], in0=mv[:, 0:H],
                                    in1=mv[:, 0:H], op=OP.mult)
            nc.vector.tensor_tensor(out=var[:], in0=mv[:, H:2 * H],
                                    in1=var[:], op=OP.subtract)
            nc.vector.tensor_scalar_add(var[:], var[:], EPS)
            sd = cpool.tile([1, H], f32)
            nc.scalar.activation(sd[:], var[:], AF.Sqrt)
            rs = cpool.tile([1, H], f32)
            nc.vector.reciprocal(rs[:], sd[:])

            scaleb = cpool.tile([1, 64], f32)
            nc.vector.tensor_tensor(out=scaleb[:, 0:H], in0=gb_s[:, 0:H],
                                    in1=rs[:], op=OP.mult)
            tmp1 = cpool.tile([1, H], f32)
            nc.vector.tensor_tensor(out=tmp1[:], in0=mv[:, 0:H],
                                    in1=scaleb[:, 0:H], op=OP.mult)
            nc.vector.tensor_tensor(out=scaleb[:, H:2 * H], in0=gb_s[:, H:2 * H],
                                    in1=tmp1[:], op=OP.subtract)

            sb_p = psmisc.tile([128, 64], f32, tag="sbp")
            nc.tensor.matmul(out=sb_p[:], lhsT=ones_row[:], rhs=scaleb[:],
                             start=True, stop=True, skip_group_check=True)
            sb = cpool.tile([128, 64], f32)
            nc.scalar.copy(sb[:], sb_p[:])

            # ---------------- pass 2: normalize + relu -----------------
            for b0 in range(0, TPC, OB):
                nb = min(OB, TPC - b0)
                ob = wpool.tile([128, OB * H], f32, tag="ob")
                for t in range(b0, b0 + nb):
                    g = raw[:, t * H:(t + 1) * H]
                    tmp = wpool.tile([128, H], f32, tag="n1")
                    nc.vector.tensor_tensor(out=tmp[:], in0=g,
                                            in1=sb[:, 0:H], op=OP.mult)
                    nc.vector.tensor_tensor(out=tmp[:], in0=tmp[:],
                                            in1=sb[:, H:2 * H], op=OP.add)
                    j = t - b0
                    nc.scalar.activation(ob[:, j * H:(j + 1) * H], tmp[:],
                                         AF.Relu)
                nc.sync.dma_start(OUT[:, b0 * H:(b0 + nb) * H],
                                  ob[:, 0:nb * H])

    nc.compile()
    _cache["nc"] = nc
    return nc


def _run_sim(nc, in_maps):
    """Local CoreSim validation path (no hardware): executes the kernel in
    the multi-core interpreter, returns a result object like the HW path."""
    import numpy as np
    from concourse.bass_interp import MultiCoreSim
    from concourse import bass_utils, mybir

    sim = MultiCoreSim(nc, num_cores=NCORES, num_workers=NCORES)
    for c in range(NCORES):
        core = sim.cores[c]
        for name, val in in_maps[c].items():
            core.tensor(name)[:] = val
        if nc.partition_id_tensor is not None:
            core.tensor(nc.partition_id_tensor.name)[:] = np.array(
                [[c]], dtype=np.uint32)
    sim.simulate()
    results = []
    for c in range(NCORES):
        outs = {}
        for alloc in nc.m.functions[0].allocations:
            if isinstance(alloc, mybir.MemoryLocationSet) and \
                    alloc.kind == "ExternalOutput":
                name = alloc.memorylocations[0].name
                outs[name] = np.array(sim.cores[c].tensor(name))
        results.append(outs)
    return bass_utils.BassKernelResults(
        results=results, instructions_and_trace=None, profile_json=None,
        exec_time_ns=None)


def _prep_idx16(inv):
    """Wrap int16 indices into the dma_gather layout: idx i at
    [i % 16, i // 16], replicated across the 8 16-partition groups."""
    pad = np.zeros(ECP, dtype=np.int16)
    pad[:EC] = inv
    w = np.ascontiguousarray(pad.reshape(ECP // 16, 16).T)   # [16, ECP//16]
    return np.ascontiguousarray(np.tile(w, (8, 1)))          # [128, ECP//16]


def kernel(h, e, feat, src_idx, dst_idx, emb_src, emb_dst, W_edge, b_edge,
           W1, b1, W2, b2, W3, b3, gamma, beta):
    global last_exec_time_ns, last_results
    import concourse.bass_utils as bass_utils

    h = np.asarray(h, np.float32)
    feat = np.asarray(feat, np.int64)
    src_idx = np.asarray(src_idx, np.int64)
    dst_idx = np.asarray(dst_idx, np.int64)
    emb_src = np.asarray(emb_src, np.float32)
    emb_dst = np.asarray(emb_dst, np.float32)
    W_edge = np.asarray(W_edge, np.float32)
    b_edge = np.asarray(b_edge, np.float32)
    W1 = np.asarray(W1, np.float32)
    b1 = np.asarray(b1, np.float32)
    W2 = np.asarray(W2, np.float32)
    b2 = np.asarray(b2, np.float32)
    W3 = np.asarray(W3, np.float32)
    b3 = np.asarray(b3, np.float32)
    gamma = np.asarray(gamma, np.float32)
    beta = np.asarray(beta, np.float32)

    # ---- host-side weight folds and node tables ----
    ES = emb_src @ W_edge[:H] + 0.5 * b_edge              # [V, H]
    ED = emb_dst @ W_edge[H:] + 0.5 * b_edge
    W1r = W1.reshape(H, H, H)                             # [i, k, d]
    W1f = np.ascontiguousarray(
        np.einsum("ikd,km->imd", W1r, W3).reshape(H, H * H)).astype(np.float32)
    Btil = np.einsum("kd,km->dm", b1.reshape(H, H), W3)   # [d, m]
    P2 = h @ W2 + b2                                      # [N, H]
    P2B = P2 @ Btil + b3                                  # [N, H]
    Hcat = np.ascontiguousarray(
        np.concatenate([h, ES[feat]], axis=1)).astype(np.float32)
    Pcat = np.ascontiguousarray(
        np.concatenate([P2, ED[feat] + P2B], axis=1)).astype(np.float32)

    gb = np.concatenate([gamma, beta]).reshape(1, 64).astype(np.float32)

    nc = _build()

    # per-core compacted tables + int16 indices + exact BN pad correction
    in_maps = []
    corr_sum = np.zeros(H, np.float64)
    corr_sq = np.zeros(H, np.float64)
    W1f3 = W1f.reshape(H, H, H)                           # [i, m, d]
    per_core = []
    for c in range(NCORES):
        sl = slice(c * EC, (c + 1) * EC)
        su, sinv = np.unique(src_idx[sl], return_inverse=True)
        du, dinv = np.unique(dst_idx[sl], return_inverse=True)
        assert len(su) <= TCAP and len(du) <= TCAP, (len(su), len(du))
        HcatC = np.zeros((TCAP, 64), np.float32)
        HcatC[:len(su)] = Hcat[su]
        PcatC = np.zeros((TCAP, 64), np.float32)
        PcatC[:len(du)] = Pcat[du]
        per_core.append((HcatC, PcatC, sinv.astype(np.int16),
                         dinv.astype(np.int16)))
        # dummy padded edge (table rows 0, 0) contribution to BN stats
        v = np.einsum("i,imd,d->m", Hcat[su[0], :H].astype(np.float64),
                      W1f3.astype(np.float64),
                      Pcat[du[0], :H].astype(np.float64)) \
            + Hcat[su[0], H:] + Pcat[du[0], H:]
        corr_sum += PAD * v
        corr_sq += PAD * v * v

    corr = np.zeros((1, 64), np.float32)
    corr[0, :H] = corr_sum
    corr[0, H:] = corr_sq

    for c in range(NCORES):
        HcatC, PcatC, sinv, dinv = per_core[c]
        in_maps.append({
            "hcatc": HcatC,
            "pcatc": PcatC,
            "w1f": W1f,
            "sidx": _prep_idx16(sinv),
            "didx": _prep_idx16(dinv),
            "gb": gb,
            "corr": corr,
        })

    _cache["last_in_maps"] = in_maps
    if os.environ.get("KERNEL_SIM", "0") == "1":
        res = _run_sim(nc, in_maps)
    else:
        trace = bool(int(os.environ.get("KERNEL_TRACE", "0")))
        res = bass_utils.run_bass_kernel_spmd(
            nc, in_maps, core_ids=list(range(NCORES)), trace=trace)
    last_results = res
    last_exec_time_ns = res.exec_time_ns

    outs = []
    for c in range(NCORES):
        o = res.results[c]["out"].reshape(128, TPC, H)
        outs.append(o.transpose(1, 0, 2).reshape(ECP, H)[:EC])
    return np.ascontiguousarray(np.concatenate(outs, axis=0))
